# revision 1
# baseline (speedup 1.0000x reference)
"""Trainium2 Bass kernel for nn_Attention_22179211117150 (sparse axial attention).

Strategy (8 NeuronCores, zero collectives):
  - Attention is axial: tokens only attend within their own frame (N=1024
    tokens per frame, F=16 frames). Shard the token axis by contiguous
    frames: 2 frames per core. QKV projection, attention, and the output
    projection are then fully local per core; weights are replicated.
  - Sparsity: the padding mask zeroes ~half the keys identically for every
    frame/head. Keys/values are compressed on the host to the kept positions
    (k/v projections, sim, softmax and attn@v all shrink ~2x).
  - All matmuls run in bf16 (f32 psum accumulation); softmax in f32.
  - Transposed dataflow: x is fed pre-transposed (xT), projections produce
    qT/kT [d, tokens] directly, sim is computed as simT [keys, queries]
    (keys on partitions), exp on ScalarE with a per-partition bias that
    zeroes padded key rows, the diagonal mask is a narrow band multiply on
    VectorE, attn@v consumes E^T directly with a ones-column appended to v
    so softmax denominators fall out of the same matmul, and the final
    projection consumes aoT [hd, tokens] with no transposes anywhere.
"""
import numpy as np
import ml_dtypes
from contextlib import ExitStack

import concourse.bass as bass
import concourse.mybir as mybir
import concourse.tile as tile
from concourse import bacc
from concourse.bass_utils import run_bass_kernel_spmd

dt = mybir.dt
AF = mybir.ActivationFunctionType
bf16 = ml_dtypes.bfloat16

B, F, N, H, D, DIM = 1, 16, 1024, 8, 64, 512
NCORES = 8
FPC = F // NCORES          # frames per core
T = FPC * N                # tokens per core
NEG = -1.0e9

TRACE = False              # set True (e.g. from test.py) to capture a profile
LAST = {}                  # exec_time_ns etc. from the last traced run

_nc_cache = {}


def _windows(total, size):
    out = []
    o = 0
    while o < total:
        out.append((o, min(size, total - o)))
        o += size
    return out


def _build(njt, diag, band_lo, band_w):
    nkp = njt * 128
    KV = FPC * nkp                     # kv rows per core (both frames, padded)
    nc = bacc.Bacc("TRN2", target_bir_lowering=False, debug=False,
                   num_devices=NCORES)

    xT_d = nc.declare_dram_parameter("xT", [128, 4 * T], dt.bfloat16, isOutput=False)
    xkvT_d = nc.declare_dram_parameter("xkvT", [128, 4 * KV], dt.bfloat16, isOutput=False)
    wq_d = nc.declare_dram_parameter("wq", [128, 4 * 512], dt.bfloat16, isOutput=False)
    wk_d = nc.declare_dram_parameter("wk", [128, 4 * 512], dt.bfloat16, isOutput=False)
    wv_d = nc.declare_dram_parameter("wv", [128, 4 * 520], dt.bfloat16, isOutput=False)
    wo_d = nc.declare_dram_parameter("wo", [128, 4 * 512], dt.bfloat16, isOutput=False)
    eb_d = nc.declare_dram_parameter("eb", [128, njt], dt.float32, isOutput=False)
    if diag:
        mmb_d = nc.declare_dram_parameter("mmb", [128, njt * band_w], dt.bfloat16,
                                          isOutput=False)
    out_d = nc.declare_dram_parameter("out", [T, DIM], dt.float32, isOutput=True)

    with tile.TileContext(nc) as tc, ExitStack() as ctx:
        consts = ctx.enter_context(tc.tile_pool(name="consts", bufs=1))
        work = ctx.enter_context(tc.tile_pool(name="work", bufs=1))
        etp = ctx.enter_context(tc.tile_pool(name="etp", bufs=22))
        smallp = ctx.enter_context(tc.tile_pool(name="small", bufs=6))
        outp = ctx.enter_context(tc.tile_pool(name="outp", bufs=3))
        dramp = ctx.enter_context(tc.tile_pool(name="dramp", bufs=8, space="DRAM"))
        psb = ctx.enter_context(tc.tile_pool(name="psb", bufs=2, space="PSUM"))
        pss = ctx.enter_context(tc.tile_pool(name="pss", bufs=4, space="PSUM"))

        def load(d, shape, dtype, tag, split=1, eng=None):
            eng = eng or nc.sync
            t = consts.tile(shape, dtype, tag=tag, name=tag)
            n = shape[1]
            step = n // split
            for o in range(0, n, step):
                eng.dma_start(t[:, o:o + step], d[:, o:o + step])
            return t

        # weights first: they're small and gate the first projection matmuls
        wq = load(wq_d, [128, 4 * 512], dt.bfloat16, "wq")
        wk = load(wk_d, [128, 4 * 512], dt.bfloat16, "wk", eng=nc.scalar)
        wv = load(wv_d, [128, 4 * 520], dt.bfloat16, "wv", eng=nc.scalar)
        wo = load(wo_d, [128, 4 * 512], dt.bfloat16, "wo")
        eb = load(eb_d, [128, njt], dt.float32, "eb", eng=nc.scalar)
        if diag:
            mmb = load(mmb_d, [128, njt * band_w], dt.bfloat16, "mmb", eng=nc.scalar)
        xT = load(xT_d, [128, 4 * T], dt.bfloat16, "xT", split=4)
        xkvT = load(xkvT_d, [128, 4 * KV], dt.bfloat16, "xkvT", split=4, eng=nc.scalar)

        ones_sb = work.tile([128, 64], dt.bfloat16, tag="ones", name="ones")
        nc.vector.memset(ones_sb[:], 1.0)

        # PE warm-up burst while inputs stream in: ~5us of back-to-back
        # matmuls pushes the HAM clock gate to full rate before the real
        # projections start. Kept live via a tiny copy + DMA to dram scratch.
        warm_src = work.tile([128, 512], dt.bfloat16, tag="warmsrc", name="warmsrc")
        nc.vector.memset(warm_src[:], 0.5)
        wps = pss.tile([128, 512], dt.float32, tag="pss", name="pss_t")
        for wi in range(24):
            nc.tensor.matmul(wps[0:64, :], ones_sb[:, 0:64], warm_src[:],
                             start=(wi == 0), stop=(wi == 23))
        wsb = smallp.tile([1, 64], dt.float32, tag="warm", name="warm_t")
        nc.vector.tensor_copy(wsb[:], wps[0:1, 0:64])
        wdr = dramp.tile([1, 64], dt.float32, tag="wdr", name="wdr_t")
        nc.sync.dma_start(wdr[:], wsb[:])

        qT = [work.tile([128, T], dt.bfloat16, tag=f"qT{hp}", name=f"qT{hp}") for hp in range(4)]
        kT = [work.tile([128, KV], dt.bfloat16, tag=f"kT{hp}", name=f"kT{hp}") for hp in range(4)]
        vt = [[work.tile([128, 520], dt.bfloat16, tag=f"v{f}_{jt}", name=f"v{f}_{jt}")
               for jt in range(njt)] for f in range(FPC)]
        aoT = [work.tile([128, T], dt.bfloat16, tag=f"aoT{hp}", name=f"aoT{hp}") for hp in range(4)]

        # ---- projections: qT[hp] = (Wq chunk).T @ xT, kT likewise ----
        for hp in range(4):
            for (w0, wl) in _windows(T, 512):
                ps = pss.tile([128, 512], dt.float32, tag="pss", name="pss_t")
                for cc in range(4):
                    nc.tensor.matmul(
                        ps[:, 0:wl],
                        wq[:, cc * 512 + hp * 128: cc * 512 + hp * 128 + 128],
                        xT[:, cc * T + w0: cc * T + w0 + wl],
                        start=(cc == 0), stop=(cc == 3))
                nc.vector.tensor_copy(qT[hp][:, w0:w0 + wl], ps[:, 0:wl])
            for (w0, wl) in _windows(KV, 512):
                ps = pss.tile([128, 512], dt.float32, tag="pss", name="pss_t")
                for cc in range(4):
                    nc.tensor.matmul(
                        ps[:, 0:wl],
                        wk[:, cc * 512 + hp * 128: cc * 512 + hp * 128 + 128],
                        xkvT[:, cc * KV + w0: cc * KV + w0 + wl],
                        start=(cc == 0), stop=(cc == 3))
                nc.vector.tensor_copy(kT[hp][:, w0:w0 + wl], ps[:, 0:wl])

        # ---- v tiles [128 kv-rows, 520] with per-head ones column ----
        for f in range(FPC):
            for jt in range(njt):
                col0 = f * nkp + jt * 128
                ps = psb.tile([128, 1024], dt.float32, tag="psb", name="psb_t")
                for cc in range(4):
                    lhs = xkvT[:, cc * KV + col0: cc * KV + col0 + 128]
                    nc.tensor.matmul(ps[:, 0:512], lhs,
                                     wv[:, cc * 520: cc * 520 + 512],
                                     start=(cc == 0), stop=(cc == 3))
                    nc.tensor.matmul(ps[:, 512:520], lhs,
                                     wv[:, cc * 520 + 512: cc * 520 + 520],
                                     start=(cc == 0), stop=(cc == 3))
                nc.vector.tensor_copy(vt[f][jt][:, 0:520], ps[:, 0:520])
                v3 = vt[f][jt][:, :].rearrange("p (h c) -> p h c", c=65)
                nc.vector.memset(v3[:, :, 64:65], 1.0)

        # ---- attention, software-pipelined across (frame, head-pair) ----
        # Emit group g+1's sim matmuls before group g's attn@v so the PE
        # (in-order queue) can fill exp/mask latency with useful work.
        def emit_sims(f, hp):
            ET = {}
            for jt in range(njt):
                # interleave the two heads' matmuls so consecutive PE ops sit
                # on disjoint row groups (partitions 0-63 vs 64-127) and
                # overlap inside the array
                pss_hr = [psb.tile([128, 1024], dt.float32, tag="psb", name="psb_t")
                          for _ in (0, 1)]
                for iw in (0, 1):
                    for hr in (0, 1):
                        po = 64 * hr
                        nc.tensor.matmul(
                            pss_hr[hr][:, iw * 512: iw * 512 + 512],
                            kT[hp][po:po + 64, f * nkp + jt * 128: f * nkp + jt * 128 + 128],
                            qT[hp][po:po + 64, f * 1024 + iw * 512: f * 1024 + iw * 512 + 512],
                            start=True, stop=True)
                for hr in (0, 1):
                    et = etp.tile([128, 1024], dt.bfloat16, tag="et", name="et_t")
                    nc.scalar.activation(et[:], pss_hr[hr][:], AF.Exp,
                                         bias=eb[:, jt:jt + 1])
                    if diag:
                        lo = band_lo[jt]
                        nc.vector.tensor_mul(
                            et[:, lo:lo + band_w], et[:, lo:lo + band_w],
                            mmb[:, jt * band_w: (jt + 1) * band_w])
                    ET[(hr, jt)] = et
            return ET

        def emit_av(f, hp, ET):
            for hr in (0, 1):
                h = hp * 2 + hr
                po = 64 * hr
                # fill both query windows first so the PE queue isn't stalled
                # by the s-broadcast matmul waiting on the DVE s-copy
                ps2s = {}
                for iw in (0, 1):
                    ps2 = pss.tile([128, 512], dt.float32, tag="pss", name="pss_t")
                    for jt in range(njt):
                        nc.tensor.matmul(
                            ps2[0:65, :],
                            vt[f][jt][:, 65 * h: 65 * h + 65],
                            ET[(hr, jt)][:, iw * 512: iw * 512 + 512],
                            start=(jt == 0), stop=(jt == njt - 1))
                    s_sb = smallp.tile([128, 512], dt.bfloat16, tag="s", name="s_t")
                    nc.vector.tensor_copy(s_sb[64:65, :], ps2[64:65, :])
                    ps2s[iw] = (ps2, s_sb)
                for iw in (0, 1):
                    ps2, s_sb = ps2s[iw]
                    win = slice(f * 1024 + iw * 512, f * 1024 + iw * 512 + 512)
                    # softmax denominators live in psum partition 64. Engines
                    # can't shift partitions: broadcast the sbuf copy down to
                    # partitions 0..63 with a K=1 matmul against ones,
                    # reciprocal, then scale the unnormalized outputs.
                    psx = pss.tile([128, 512], dt.float32, tag="pss", name="pss_t")
                    nc.tensor.matmul(psx[0:64, :], ones_sb[64:65, 0:64],
                                     s_sb[64:65, :], start=True, stop=True)
                    sr = smallp.tile([64, 512], dt.float32, tag="sr", name="sr_t")
                    nc.vector.reciprocal_approx_fast(sr[:], psx[0:64, :])
                    if hr == 0:
                        nc.vector.tensor_mul(aoT[hp][0:64, win],
                                             ps2[0:64, :], sr[:])
                    else:
                        # DMA (not an engine op) moves the result to
                        # partitions 64..127 of the aoT tile.
                        sc = smallp.tile([64, 512], dt.bfloat16, tag="aosc",
                                         name="aosc_t")
                        nc.vector.tensor_mul(sc[:], ps2[0:64, :], sr[:])
                        nc.sync.dma_start(aoT[hp][64:128, win], sc[:])

        # ---- output projection: out[t, e] = aoT.T @ Wout (per frame) ----
        def emit_final(f):
            for tt in range(f * (N // 128), (f + 1) * (N // 128)):
                ps = pss.tile([128, 512], dt.float32, tag="pss", name="pss_t")
                for hp in range(4):
                    nc.tensor.matmul(ps[:],
                                     aoT[hp][:, tt * 128:(tt + 1) * 128],
                                     wo[:, hp * 512:(hp + 1) * 512],
                                     start=(hp == 0), stop=(hp == 3))
                osb = outp.tile([128, 512], dt.float32, tag="osb", name="osb_t")
                nc.scalar.copy(osb[:], ps[:])
                nc.sync.dma_start(out_d[tt * 128:(tt + 1) * 128, :], osb[:])

        groups = [(f, hp) for f in range(FPC) for hp in range(4)]
        prev = None
        for (f, hp) in groups:
            ET = emit_sims(f, hp)
            if prev is not None:
                emit_av(*prev)
                if prev[1] == 3:
                    emit_final(prev[0])
            prev = (f, hp, ET)
        emit_av(*prev)
        emit_final(prev[0])

    nc.compile()
    return nc


def _chunk_major(a):
    """[512, M] f32 -> [128, 4*M] bf16, contraction chunk-major."""
    m = a.shape[1]
    return np.ascontiguousarray(
        a.reshape(4, 128, m).transpose(1, 0, 2).reshape(128, 4 * m)).astype(bf16)


def kernel(x, W_qkv, W_out, mask, diag):
    x = np.asarray(x, dtype=np.float32).reshape(F * N, DIM)
    W_qkv = np.asarray(W_qkv, dtype=np.float32)
    W_out = np.asarray(W_out, dtype=np.float32)
    maskv = np.asarray(mask).reshape(N)
    diag = int(np.asarray(diag))

    kept = np.flatnonzero(maskv != 0)
    nk = int(kept.size)
    assert nk > 0, "all-masked input not supported"
    njt = (nk + 127) // 128
    nkp = njt * 128

    Wq = W_qkv[:, 0:512] * np.float32(D ** -0.5)
    Wk = W_qkv[:, 512:1024]
    Wv = W_qkv[:, 1024:1536]

    wq_h = _chunk_major(Wq)
    wk_h = _chunk_major(Wk)
    Wv_aug = np.zeros((512, 520), np.float32)
    for h in range(H):
        Wv_aug[:, 65 * h: 65 * h + 64] = Wv[:, 64 * h: 64 * h + 64]
    wv_h = _chunk_major(Wv_aug)
    wo_h = _chunk_major(W_out)

    eb_h = np.zeros((128, njt), np.float32)
    for jt in range(njt):
        rows = np.arange(jt * 128, jt * 128 + 128)
        eb_h[:, jt] = np.where(rows < nk, 0.0, NEG)

    if diag:
        los, ws = [], []
        for jt in range(njt):
            idx = kept[jt * 128: min(jt * 128 + 128, nk)]
            lo = int(idx.min()) & ~1
            los.append(lo)
            ws.append(int(idx.max()) + 1 - lo)
        bw = (max(ws) + 1) & ~1
        los = [min(lo, N - bw) for lo in los]
        mmb_h = np.ones((128, njt * bw), np.float32)
        for jt in range(njt):
            valid = min(128, nk - jt * 128)
            p = np.arange(valid)
            mmb_h[p, jt * bw + (kept[jt * 128: jt * 128 + valid] - los[jt])] = 0.0
        mmb_h = mmb_h.astype(bf16)
        band_lo = tuple(los)
    else:
        bw = 0
        band_lo = None
        mmb_h = None

    key = (njt, diag, bw, band_lo)
    if key not in _nc_cache:
        _nc_cache[key] = _build(njt, diag, band_lo, bw)
    nc = _nc_cache[key]

    xbf = x.astype(bf16)
    in_maps = []
    for m in range(NCORES):
        xs = xbf[m * T:(m + 1) * T]                      # [T, DIM] bf16
        xT_h = _chunk_major(np.ascontiguousarray(xs.T.astype(np.float32)))
        kvrows = np.zeros((FPC * nkp, DIM), np.float32)
        for f in range(FPC):
            kvrows[f * nkp: f * nkp + nk] = xs[f * N + kept].astype(np.float32)
        xkvT_h = _chunk_major(np.ascontiguousarray(kvrows.T))
        im = dict(xT=xT_h, xkvT=xkvT_h, wq=wq_h, wk=wk_h, wv=wv_h, wo=wo_h,
                  eb=eb_h)
        if diag:
            im["mmb"] = mmb_h
        in_maps.append(im)

    core_ids = list(range(NCORES))
    if TRACE:
        r = run_bass_kernel_spmd(nc, in_maps, core_ids, trace=True)
        LAST["exec_time_ns"] = r.exec_time_ns
        LAST["results"] = r
        results = r.results
    else:
        results = None
        for attempt in range(3):
            try:
                results = run_bass_kernel_spmd(nc, in_maps, core_ids).results
                break
            except Exception:
                if attempt == 2:
                    raise
                import time as _time
                _time.sleep(2.0)

    out = np.concatenate([np.asarray(results[m]["out"]) for m in range(NCORES)],
                         axis=0)
    return out.reshape(B, F * N, DIM).astype(np.float32)

